# revision 14
# baseline (speedup 1.0000x reference)
"""Llama-3.2 attention block (T=2048, H=2048, 32 q heads / 8 kv heads, d=64)
as a Bass/Tile kernel on 8 Trainium2 NeuronCores.

Sharding: tensor-parallel over heads. Core c owns q heads 4c..4c+3 and kv
head c (the GQA group). Each core projects its QKV shard over the full
sequence, applies RoPE, runs causal attention for its 4 heads. After EACH
head finishes, a per-head AllToAll ([8 dst, 64, 256] bf16) fires so the
collectives pipeline behind the remaining heads' compute; core c ends up
with the full 2048-feature attention output for its 256 sequence rows, then
o_proj runs in three passes (k0..7 gated on heads 0-1's collectives, k8..11
on head 2's, k12..15 on head 3's) and writes a [256, 2048] fp32 output
slice.

Layouts on device (bf16 matmul inputs, fp32 accumulation):
  - hidden and weights are pre-transposed AND pre-tiled on host so the
    contraction dim (hidden) lands on SBUF partitions and each partition's
    k-chunks are contiguous in DRAM — DMA trigger instructions cost ~600ns
    of serial engine time each, so chunk loads are batched into few
    triggers with multi-KB lines (fine-grained only for the first chunks
    that gate compute start).
  - QKV is produced transposed: q/k/v as [feat, seq] tiles. RoPE is applied
    in this layout: out = x * cos + (P @ x) * sin, where P is the
    rotate-half permutation done on the tensor engine.
  - q_t[m] holds heads 2m/2m+1 on partitions 0:64/64:128. Score matmuls
    contract over K=128 against zero-padded k tiles: k_t (k at 0:64, zeros
    above) serves even heads, k2_t (zeros, then k at 64:128) serves odd
    heads — the dead half of q contracts against zeros. K=64 matmuls
    measure ~2.3x slower per moving column on TRN2, hence the padding.
  - scores are computed transposed (scoresT[k, q]) so softmax's exp runs on
    the scalar engine and P@V consumes probsT directly as the moving
    operand; the softmax denominator rides along as a ones-column appended
    to V. No max-subtraction is needed: |scores| <= ~20 for this problem,
    safely inside fp32 exp range. 1/denom is broadcast across partitions
    with a K=1 ones matmul on the tensor engine.
  - cc-dependent loads (o_proj's lo chunks) ride the GPSIMD software-DGE
    queue, which is already serialized with the collectives — putting them
    on the sync or vector queues head-of-line blocks the staging/probs
    pipelines for the collective's full skew-absorption latency. Head 3's
    collective trigger is emitted BEFORE head 2's lo loads so it is not
    head-of-line blocked behind cc2's completion.
"""

import os
import sys
import types

import numpy as np
import ml_dtypes

T = 2048
HID = 2048
NH = 32
NKV = 8
D = 64
NCORES = 8
HPC = NH // NCORES        # q heads per core = 4
FPC = HPC * D             # attention feats per core = 256
SPC = T // NCORES         # seq chunk per core after AllToAll = 256
QKV_F = FPC + 2 * D       # per-core qkv proj feats = 384
ROPE_THETA = 500000.0
SCALE = float(D) ** -0.5

_CACHE = {}


def _ensure_trace_hooks():
    """Register the NTFF profiling hook that the stub antenv package lacks."""
    if "antenv.axon_hooks" in sys.modules:
        return
    try:
        import antenv
    except ImportError:
        return
    hooks = types.ModuleType("antenv.axon_hooks")
    holder = [None]
    hooks.set_axon_ntff_profile_hook = lambda h: holder.__setitem__(0, h)
    hooks.get_axon_ntff_profile_hook = lambda: holder[0]
    antenv.axon_hooks = hooks
    sys.modules["antenv.axon_hooks"] = hooks
    try:
        from trn_agent_boot.trn_boot import _ntff_profile_via_ctypes

        hook = _ntff_profile_via_ctypes("/opt/axon/libaxon_pjrt.so")
        if hook is not None:
            hooks.set_axon_ntff_profile_hook(hook)
    except Exception:
        pass


def _build():
    from contextlib import ExitStack

    from concourse import bacc
    import concourse.mybir as mybir
    import concourse.tile as tile
    from concourse.bass import ts
    from concourse.tile import add_dep_helper

    f32 = mybir.dt.float32
    bf16 = mybir.dt.bfloat16
    AF = mybir.ActivationFunctionType
    OP = mybir.AluOpType

    # stream_shuffle permutes partitions WITHIN each 32-partition quadrant
    # (mask[i] = source partition within the quadrant). The rope pairs are
    # interleaved host-side (per-head order [x1a, x2a, x1b, x2b] in 16-blocks)
    # so the rotate-half partner always sits 16 partitions away in the same
    # quadrant: out[p] = in[p ^ 16].
    SHUF_MASK = [i ^ 16 for i in range(32)]
    KO = HID // 128           # 16 contraction chunks
    NQ = T // 512             # 4 seq chunks of 512
    NB = T // 128             # 16 k blocks of 128

    nc = bacc.Bacc("TRN2", target_bir_lowering=False, debug=False, num_devices=NCORES)

    hT = nc.dram_tensor("hT", [T // 512, 128, HID // 128, 512], bf16, kind="ExternalInput")
    # weights pre-tiled host-side to [128 partitions, KO * feat] so a
    # multi-k-chunk DMA reads contiguous multi-KB lines per partition
    wT = nc.dram_tensor("wT", [128, KO * QKV_F], bf16, kind="ExternalInput")
    cosf = nc.dram_tensor("cosf", [128, T], bf16, kind="ExternalInput")
    sinf = nc.dram_tensor("sinf", [128, T], bf16, kind="ExternalInput")
    # perm / ident / tri / ones packed into one tensor = one DMA trigger
    misc = nc.dram_tensor("misc", [128, 4, 128], bf16, kind="ExternalInput")
    woT = nc.dram_tensor("woT", [128, KO * HID], bf16, kind="ExternalInput")
    out = nc.dram_tensor("out", [SPC, HID], f32, kind="ExternalOutput")
    # one AllToAll per head: [dst core, head feats, seq chunk]
    a2a_in = [
        nc.dram_tensor(f"a2a_in{g}", [NCORES, D, SPC], bf16) for g in range(HPC)
    ]
    a2a_out = [
        nc.dram_tensor(f"a2a_out{g}", [NCORES, D, SPC], bf16) for g in range(HPC)
    ]

    with tile.TileContext(nc) as tc, ExitStack() as ctx:
        consts = ctx.enter_context(tc.tile_pool(name="consts", bufs=1))
        persist = ctx.enter_context(tc.tile_pool(name="persist", bufs=1))

        # first QKV matmuls are gated on wt + the first hT chunk; load those
        # first, finest-grained up front so compute starts as early as
        # possible, coarser (fatter-lined, fewer triggers) after
        wt_t = consts.tile([128, KO, QKV_F], bf16, tag="wt")
        ht0_t = consts.tile([128, KO, 512], bf16, tag="ht0")
        nc.sync.dma_start(wt_t[:, 0, :], wT.ap()[:, 0:QKV_F])
        # first hT chunk rides the (otherwise idle) scalar queue so the two
        # gating transfers start in parallel instead of trigger-serialized
        nc.scalar.dma_start(ht0_t[:, 0:1, :], hT.ap()[0, :, 0:1, :])
        nc.sync.dma_start(wt_t[:, 1:4, :], wT.ap()[:, QKV_F:4 * QKV_F])
        nc.sync.dma_start(ht0_t[:, 1:4, :], hT.ap()[0, :, 1:4, :])
        nc.sync.dma_start(wt_t[:, 4:8, :], wT.ap()[:, 4 * QKV_F:8 * QKV_F])
        nc.sync.dma_start(ht0_t[:, 4:8, :], hT.ap()[0, :, 4:8, :])
        nc.sync.dma_start(wt_t[:, 8:16, :], wT.ap()[:, 8 * QKV_F:16 * QKV_F])
        nc.sync.dma_start(ht0_t[:, 8:12, :], hT.ap()[0, :, 8:12, :])
        nc.sync.dma_start(ht0_t[:, 12:16, :], hT.ap()[0, :, 12:16, :])
        cos_t = consts.tile([128, T], bf16, tag="cos")
        nc.sync.dma_start(cos_t, cosf.ap())
        sin_t = consts.tile([128, T], bf16, tag="sin")
        nc.sync.dma_start(sin_t, sinf.ap())
        misc_t = consts.tile([128, 4, 128], bf16, tag="misc")
        nc.sync.dma_start(misc_t, misc.ap())
        # hT chunks n=1..3 trigger up-front (3 pool bufs, so no buffer-free
        # waits) — in the n-loop they would queue behind the rope-gated
        # k2/vlo copies and head-of-line block the input feed
        ht_tiles = [None] * NQ
        perm_t = misc_t[:, 0, :]
        ident_t = misc_t[:, 1, :]
        tri_t = misc_t[:, 2, :]
        ones_t = misc_t[:, 3, :]

        # Persistent activation tiles (live across phases A/B). q_t[m] holds
        # heads 2m (partitions 0:64) and 2m+1 (64:128); scores for the even
        # head contract against k_t (k at 0:64, zeros at 64:128), the odd head
        # against k2_t (zeros at 0:64, k at 64:128) — no per-head q copies.
        q_t = [persist.tile([128, T], bf16, tag=f"q{p}", name=f"q{p}") for p in range(2)]
        k_t = persist.tile([128, T], bf16, tag="kt")
        k2_t = persist.tile([128, T], bf16, tag="k2t")
        vlo_t = persist.tile([64, T], bf16, tag="vlo")
        vaug_t = persist.tile([128, KO, D + 1], bf16, tag="vaug")

        # zero the K-padding rows once (on gpsimd: idle at startup, and the
        # vector engine is on the QKV critical path)
        nc.gpsimd.memset(k_t[64:128, :], 0.0)
        nc.gpsimd.memset(k2_t[0:64, :], 0.0)

        # ---- Phase A: QKV projection + RoPE (outputs transposed [feat, seq]) ----
        with nc.named_scope("qkv"):
            with (
                tc.tile_pool(name="htp", bufs=3) as ht_pool,
                tc.tile_pool(name="atmp", bufs=3) as atmp,
                tc.tile_pool(name="psA", bufs=6, space="PSUM") as psA,
                tc.tile_pool(name="psV", bufs=2, space="PSUM") as psV,
            ):
                nc.gpsimd.memset(vaug_t[:, :, D:D + 1], 1.0)
                ht_tiles[0] = ht0_t
                for n in range(1, NQ):
                    ht_tiles[n] = ht_pool.tile(
                        [128, KO, 512], bf16, tag="ht", name=f"ht{n}"
                    )
                    for k4 in range(4):
                        nc.sync.dma_start(
                            ht_tiles[n][:, ts(k4, 4), :], hT.ap()[n, :, ts(k4, 4), :]
                        )
                for n in range(NQ):
                    ht_t = ht_tiles[n]
                    pending = None
                    for m in range(3):
                        pq = psA.tile([128, 512], f32, tag="pq")
                        for k in range(KO):
                            nc.tensor.matmul(
                                pq,
                                wt_t[:, k, ts(m, 128)],
                                ht_t[:, k, :],
                                start=(k == 0),
                                stop=(k == KO - 1),
                            )
                        # PSUM->SBUF cast on the scalar engine: it is idle all
                        # through QKV, while the vector queue carries the rope
                        # chain — keeps psA recycling off the vector backlog
                        xb = atmp.tile([128, 512], bf16, tag="xb")
                        nc.scalar.copy(xb, pq)
                        if pending is not None:
                            pending()
                            pending = None
                        if m < 2:
                            def rope_q(m=m, n=n, xb=xb):
                                # two q heads: rotate-half via DVE stream_shuffle
                                # (swaps 32-partition blocks within each 64)
                                psh = atmp.tile([128, 512], bf16, tag="psh", name="psh")
                                nc.vector.stream_shuffle(psh, xb, SHUF_MASK)
                                t1 = atmp.tile([128, 512], f32, tag="t1", name="t1")
                                nc.vector.tensor_tensor(t1, xb, cos_t[:, ts(n, 512)], OP.mult)
                                t2 = atmp.tile([128, 512], f32, tag="t2", name="t2")
                                nc.vector.tensor_tensor(t2, psh, sin_t[:, ts(n, 512)], OP.mult)
                                nc.vector.tensor_tensor(q_t[m][:, ts(n, 512)], t1, t2, OP.add)
                            pending = rope_q
                        else:
                            def rope_kv(n=n, xb=xb):
                                # k head on partitions 0:64 (rope), v on 64:128
                                psh = atmp.tile([128, 512], bf16, tag="psh", name="psh")
                                nc.vector.stream_shuffle(psh, xb, SHUF_MASK)
                                t1 = atmp.tile([128, 512], f32, tag="t1", name="t1")
                                nc.vector.tensor_tensor(
                                    t1[0:64, :], xb[0:64, :], cos_t[0:64, ts(n, 512)], OP.mult
                                )
                                t2 = atmp.tile([128, 512], f32, tag="t2", name="t2")
                                nc.vector.tensor_tensor(
                                    t2[0:64, :], psh[0:64, :], sin_t[0:64, ts(n, 512)], OP.mult
                                )
                                nc.vector.tensor_tensor(
                                    k_t[0:64, ts(n, 512)], t1[0:64, :], t2[0:64, :], OP.add
                                )
                                # odd heads' k copy (partition remap via DMA)
                                nc.sync.dma_start(
                                    k2_t[64:128, ts(n, 512)], k_t[0:64, ts(n, 512)]
                                )
                                # v slice to partitions 0:64 via DMA (partition remap)
                                nc.sync.dma_start(vlo_t[:, ts(n, 512)], xb[64:128, :])
                                for j in range(4 * n, 4 * n + 4):
                                    pv = psV.tile([128, D], bf16, tag="pv", name="pv")
                                    nc.tensor.transpose(
                                        pv, vlo_t[:, ts(j, 128)], ident_t[0:64, 0:64]
                                    )
                                    nc.vector.tensor_copy(vaug_t[:, j, 0:D], pv)
                            pending = rope_kv
                    if pending is not None:
                        pending()



        wo_t = consts.tile([128, KO, HID], bf16, tag="wo")
        lo_t = persist.tile([128, KO, SPC], bf16, tag="lo", name="lo")
        # w_o streams in right after the QKV input traffic drains; it must be
        # ahead of the attention staging writes so the collectives' completion
        # counters never queue behind these 8MB of transfers
        for k4 in range(4):
            nc.sync.dma_start(
                wo_t[:, ts(k4, 4), :], woT.ap()[:, 4 * k4 * HID:4 * (k4 + 1) * HID]
            )

        # ---- Phase B: causal attention, 4 heads, scoresT layout ----
        a2a_dmas = []
        ccs = []

        def _emit_cc(g):
            cc = nc.gpsimd.collective_compute(
                "AllToAll",
                OP.bypass,
                replica_groups=[list(range(NCORES))],
                ins=[a2a_in[g].ap()],
                outs=[a2a_out[g].ap()],
            )
            for gg, dd in a2a_dmas:
                if gg == g:
                    add_dep_helper(cc.ins, dd.ins, sync=True, reason="cc waits a2a stage-in")
            return cc

        def _emit_lo(g):
            # o_proj k-chunks stream in as soon as the collective lands. These
            # ride the GPSIMD software-DGE queue: it is already serialized
            # with the collectives, so waiting on cc_g there never blocks the
            # staging/probs pipelines (the sync + vector queues would suffer
            # head-of-line blocking instead). Two batched triggers per head
            # (even/odd src cores) instead of four.
            v = a2a_out[g].ap().rearrange("(j two) p s -> two p j s", two=2)
            for half in range(2):
                dl = nc.gpsimd.dma_start(
                    lo_t[64 * half:64 * half + 64, ts(g, 4), :], v[half]
                )
                add_dep_helper(dl.ins, ccs[g].ins, sync=True, reason="o_proj waits AllToAll")

        with nc.named_scope("attn"):
            with (
                tc.tile_pool(name="probs", bufs=2) as probs_pool,
                tc.tile_pool(name="btmp", bufs=4) as btmp,
                tc.tile_pool(name="psS", bufs=2, space="PSUM") as psS,
                tc.tile_pool(name="psO", bufs=2, space="PSUM") as psO,
                tc.tile_pool(name="psB", bufs=2, space="PSUM") as psB,
            ):
                # the normalize chain (dbc copy -> pb broadcast matmul ->
                # recip -> oth -> staging) of chunk (h, i) is emitted DEFERRED,
                # two score-chunks into the next (h, i): the pb matmul rides
                # the in-order PE queue, so emitted inline it head-of-line
                # blocks the next chunk's scores while the vector engine
                # finishes dbc. Collective/lo emission moves with it.
                pending_norm = [None]
                cc_done = [False] * HPC

                def flush_norm():
                    if pending_norm[0] is not None:
                        pending_norm[0]()
                        pending_norm[0] = None

                def emit_cc_and_lo(g):
                    ccs.append(_emit_cc(g))
                    cc_done[g] = True
                    if g <= 1:
                        _emit_lo(g)
                    elif g == 3:
                        _emit_lo(2)
                        _emit_lo(3)

                for h in range(HPC):
                    for i in range(NQ):
                        nj = 4 * i + 4
                        pr = probs_pool.tile([128, NB, 512], bf16, tag="pr")
                        po = psO.tile([D + 1, 512], f32, tag="po")
                        # every adjacent block pair shares one psum tile + exp
                        # call (the exp over a diagonal pair spans from the
                        # earlier block's causal offset; the later block's
                        # columns below its own offset hold exp(stale psum) —
                        # bounded, finite, and never read by P@V). P@V is
                        # lagged two chunks behind the scores so the PE never
                        # stalls on the exp chain.
                        chunks = [[j, j + 1] for j in range(0, nj, 2)]

                        kh_t = k_t if h % 2 == 0 else k2_t
                        qm_t = q_t[h // 2]

                        def emit_scores(js, i=i, pr=pr, kh_t=kh_t, qm_t=qm_t):
                            offs = [max(0, jj - 4 * i) * 128 for jj in js]
                            pss = psS.tile([128, 2, 512], f32, tag="pss", name="pss")
                            for u in range(2):
                                nc.tensor.matmul(
                                    pss[:, u, offs[u]:512],
                                    kh_t[:, ts(js[u], 128)],
                                    qm_t[:, i * 512 + offs[u]:(i + 1) * 512],
                                    start=True, stop=True,
                                )
                            nc.scalar.activation(
                                pr[:, js[0]:js[0] + 2, offs[0]:512],
                                pss[:, :, offs[0]:512],
                                AF.Exp, scale=SCALE,
                            )
                            for u in range(2):
                                r = js[u] - 4 * i
                                if r >= 0:  # block overlapping the causal diagonal
                                    off = offs[u]
                                    nc.vector.tensor_tensor(
                                        pr[:, js[u], off:off + 128],
                                        pr[:, js[u], off:off + 128],
                                        tri_t, OP.mult,
                                    )

                        def emit_pv(js, i=i, pr=pr, po=po, nj=nj):
                            for jj in js:
                                off = max(0, jj - 4 * i) * 128
                                nc.tensor.matmul(
                                    po[:, off:512], vaug_t[:, jj, :], pr[:, jj, off:512],
                                    start=(jj == 0), stop=(jj == nj - 1),
                                )

                        LAG = 2
                        for ci, ch in enumerate(chunks):
                            emit_scores(ch)
                            if ci == 1:
                                flush_norm()
                                if i == 0 and h > 0 and not cc_done[h - 1]:
                                    emit_cc_and_lo(h - 1)
                            if ci >= LAG:
                                emit_pv(chunks[ci - LAG])
                        for ci in range(max(0, len(chunks) - LAG), len(chunks)):
                            emit_pv(chunks[ci])

                        def norm(h=h, i=i, po=po):
                            # normalize: oT[f, q] = po[f, q] / den[q]; den row
                            # broadcast across partitions via a K=1 ones
                            # matmul, then 1/x on DVE
                            dbc = btmp.tile([D + 1, 512], bf16, tag="dbc")
                            nc.vector.tensor_copy(dbc[D:D + 1, :], po[D:D + 1, :])
                            pb = psB.tile([D, 512], f32, tag="pb")
                            nc.tensor.matmul(
                                pb, ones_t[D:D + 1, 0:D], dbc[D:D + 1, :],
                                start=True, stop=True,
                            )
                            rbs = btmp.tile([D, 512], f32, tag="rbs")
                            nc.vector.reciprocal_approx_fast(out=rbs, in_=pb)
                            oth = btmp.tile([D, 512], bf16, tag="oth")
                            nc.vector.tensor_tensor(oth, po[0:D, :], rbs, OP.mult)
                            for half in range(2):
                                dd = nc.sync.dma_start(
                                    a2a_in[h].ap()[2 * i + half, :, :],
                                    oth[:, ts(half, 256)],
                                )
                                a2a_dmas.append((h, dd))

                        pending_norm[0] = norm
                flush_norm()
                emit_cc_and_lo(3)

        # ---- Phase D: o_proj for this core's 256 seq rows ----
        # Three k-passes matched to collective arrival: k0..7 (heads 0-1,
        # landed long before attention drains), k8..11 (head 2), k12..15
        # (head 3, the only chunks on the final collective's critical path).
        # Output stores are batched per group pair ([128, 1024] fp32).
        with nc.named_scope("oproj"):
            with (
                tc.tile_pool(name="dtmp", bufs=2) as dtmp,
                tc.tile_pool(name="psD", bufs=1, space="PSUM") as psD,
            ):
                groups = [(m, e4) for m in range(SPC // 128) for e4 in range(HID // 512)]
                psos = [
                    psD.tile([128, 512], f32, tag=f"pso{g}", name=f"pso{g}")
                    for g in range(len(groups))
                ]
                for ka, kb in ((0, 8), (8, 12)):
                    for g, (m, e4) in enumerate(groups):
                        for k in range(ka, kb):
                            nc.tensor.matmul(
                                psos[g],
                                lo_t[:, k, ts(m, 128)],
                                wo_t[:, k, ts(e4, 512)],
                                start=(k == 0),
                                stop=False,
                            )
                ob = None
                for g, (m, e4) in enumerate(groups):
                    for k in range(12, KO):
                        nc.tensor.matmul(
                            psos[g],
                            lo_t[:, k, ts(m, 128)],
                            wo_t[:, k, ts(e4, 512)],
                            start=False,
                            stop=(k == KO - 1),
                        )
                    if g < len(groups) - 2:
                        if g % 2 == 0:
                            ob = dtmp.tile([128, 1024], f32, tag="ob")
                        nc.vector.tensor_copy(ob[:, 512 * (g % 2):512 * (g % 2) + 512], psos[g])
                        if g % 2 == 1:
                            nc.sync.dma_start(
                                out.ap()[ts(m, 128), 1024 * ((e4 - 1) // 2):1024 * ((e4 - 1) // 2) + 1024],
                                ob,
                            )
                    else:
                        # last pair: per-512 stores with the copies split
                        # across scalar+vector so the final transfer starts
                        # one copy earlier and pipelines behind the other
                        obh = dtmp.tile([128, 512], f32, tag="obh")
                        if g % 2 == 0:
                            nc.scalar.copy(obh, psos[g])
                        else:
                            nc.vector.tensor_copy(obh, psos[g])
                        nc.sync.dma_start(
                            out.ap()[ts(m, 128), 512 * e4:512 * e4 + 512], obh
                        )

    nc.compile()
    return nc


def _get_nc():
    if "nc" not in _CACHE:
        _CACHE["nc"] = _build()
    return _CACHE["nc"]


def _host_prep(hidden_states, positions, w_qkv, w_o):
    bf16 = ml_dtypes.bfloat16
    hTb = np.ascontiguousarray(hidden_states.astype(np.float32).T).astype(bf16)
    # pretile to [n, p, ko, s] so each 512-seq chunk is one contiguous DMA
    hTt = np.ascontiguousarray(
        hTb.reshape(HID // 128, 128, T // 512, 512).transpose(2, 1, 0, 3)
    )
    woTb = np.ascontiguousarray(w_o.astype(np.float32).T).astype(bf16)
    # o_proj contraction order matches the per-head AllToAll arrival order:
    # head h of every core, h = 0..3
    rows = np.concatenate(
        [
            (np.arange(NCORES)[:, None] * FPC + h * D + np.arange(D)[None, :]).reshape(-1)
            for h in range(HPC)
        ]
    )
    woTb = woTb[rows]
    # pre-tile to [128, KO * HID]: partition p holds its k-chunks contiguously
    KO = HID // 128
    woTb = np.ascontiguousarray(
        woTb.reshape(KO, 128, HID).transpose(1, 0, 2).reshape(128, KO * HID)
    )

    inv = 1.0 / (ROPE_THETA ** (np.arange(0, D, 2, dtype=np.float32) / D))  # [32]
    ang = positions.astype(np.float32)[:, None] * inv[None, :]              # [T, 32]
    cos = np.cos(ang).T  # [32, T]
    sin = np.sin(ang).T
    # rope-pair interleave: position p holds original feature P64[p % 64]
    P64 = np.r_[0:16, 32:48, 16:32, 48:64]
    p = np.arange(128)
    fr = P64[p % D] % (D // 2)
    sgn = np.where(P64[p % D] < (D // 2), -1.0, 1.0).astype(np.float32)
    cosf = np.ascontiguousarray(cos[fr]).astype(bf16)                 # [128, T]
    sinf = np.ascontiguousarray(sin[fr] * sgn[:, None]).astype(bf16)  # [128, T]

    partner = p ^ 16
    perm = np.zeros((128, 128), dtype=np.float32)
    perm[p, partner] = 1.0
    ident = np.eye(128, dtype=np.float32)
    tri = (np.arange(128)[None, :] >= np.arange(128)[:, None]).astype(np.float32)
    ones_m = np.ones((128, 128), dtype=np.float32)
    misc = np.ascontiguousarray(
        np.stack([perm, ident, tri, ones_m], axis=1)
    ).astype(bf16)  # [128, 4, 128]

    q_size = NH * D
    kv_size = NKV * D
    in_maps = []
    for c in range(NCORES):
        wq = w_qkv[c * FPC:(c + 1) * FPC]
        wq = wq.reshape(HPC, D, HID)[:, P64, :].reshape(FPC, HID)
        wk = w_qkv[q_size + c * D:q_size + (c + 1) * D][P64]
        wv = w_qkv[q_size + kv_size + c * D:q_size + kv_size + (c + 1) * D]
        wTc = np.ascontiguousarray(
            np.concatenate([wq, wk, wv], axis=0).astype(np.float32).T
        ).astype(bf16)  # [HID, QKV_F]
        # pre-tile to [128, KO * QKV_F]
        wTc = np.ascontiguousarray(
            wTc.reshape(KO, 128, QKV_F).transpose(1, 0, 2).reshape(128, KO * QKV_F)
        )
        in_maps.append(
            {
                "hT": hTt,
                "wT": wTc,
                "cosf": cosf,
                "sinf": sinf,
                "misc": misc,
                "woT": woTb,
            }
        )
    return in_maps


def run(inputs, trace=False):
    """Run on 8 NeuronCores; returns (full_output, BassKernelResults)."""
    if trace:
        _ensure_trace_hooks()
    from concourse import bass_utils

    if trace:
        bass_utils.upload_artifacts = lambda tmpdir: tmpdir
    nc = _get_nc()
    in_maps = _host_prep(
        np.asarray(inputs["hidden_states"]),
        np.asarray(inputs["positions"]),
        np.asarray(inputs["w_qkv"]),
        np.asarray(inputs["w_o"]),
    )
    res = bass_utils.run_bass_kernel_spmd(
        nc, in_maps, core_ids=list(range(NCORES)), trace=trace
    )
    full = np.concatenate(
        [res.results[c]["out"] for c in range(NCORES)], axis=0
    ).astype(np.float32)
    return full, res


def kernel(**inputs) -> np.ndarray:
    trace = bool(os.environ.get("KERNEL_TRACE"))
    full, _ = run(inputs, trace=trace)
    return full


# revision 15
# speedup vs baseline: 1.0291x; 1.0291x over previous
"""Llama-3.2 attention block (T=2048, H=2048, 32 q heads / 8 kv heads, d=64)
as a Bass/Tile kernel on 8 Trainium2 NeuronCores.

Sharding: tensor-parallel over heads. Core c owns q heads 4c..4c+3 and kv
head c (the GQA group). Each core projects its QKV shard over the full
sequence, applies RoPE, runs causal attention for its 4 heads. After EACH
head finishes, a per-head AllToAll ([8 dst, 64, 256] bf16) fires so the
collectives pipeline behind the remaining heads' compute; core c ends up
with the full 2048-feature attention output for its 256 sequence rows, then
o_proj runs in three passes (k0..7 gated on heads 0-1's collectives, k8..11
on head 2's, k12..15 on head 3's) and writes a [256, 2048] fp32 output
slice.

Layouts on device (bf16 matmul inputs, fp32 accumulation):
  - hidden and weights are pre-transposed AND pre-tiled on host so the
    contraction dim (hidden) lands on SBUF partitions and each partition's
    k-chunks are contiguous in DRAM — DMA trigger instructions cost ~600ns
    of serial engine time each, so chunk loads are batched into few
    triggers with multi-KB lines (fine-grained only for the first chunks
    that gate compute start).
  - QKV is produced transposed: q/k/v as [feat, seq] tiles. RoPE is applied
    in this layout: out = x * cos + (P @ x) * sin, where P is the
    rotate-half permutation done on the tensor engine.
  - q_t[m] holds heads 2m/2m+1 on partitions 0:64/64:128. Score matmuls
    contract over K=128 against zero-padded k tiles: k_t (k at 0:64, zeros
    above) serves even heads, k2_t (zeros, then k at 64:128) serves odd
    heads — the dead half of q contracts against zeros. K=64 matmuls
    measure ~2.3x slower per moving column on TRN2, hence the padding.
  - scores are computed transposed (scoresT[k, q]) so softmax's exp runs on
    the scalar engine and P@V consumes probsT directly as the moving
    operand; the softmax denominator rides along as a ones-column appended
    to V. No max-subtraction is needed: |scores| <= ~20 for this problem,
    safely inside fp32 exp range. 1/denom is broadcast across partitions
    with a K=1 ones matmul on the tensor engine.
  - cc-dependent loads (o_proj's lo chunks) ride the GPSIMD software-DGE
    queue, which is already serialized with the collectives — putting them
    on the sync or vector queues head-of-line blocks the staging/probs
    pipelines for the collective's full skew-absorption latency. Head 3's
    collective trigger is emitted BEFORE head 2's lo loads so it is not
    head-of-line blocked behind cc2's completion.
"""

import os
import sys
import types

import numpy as np
import ml_dtypes

T = 2048
HID = 2048
NH = 32
NKV = 8
D = 64
NCORES = 8
HPC = NH // NCORES        # q heads per core = 4
FPC = HPC * D             # attention feats per core = 256
SPC = T // NCORES         # seq chunk per core after AllToAll = 256
QKV_F = FPC + 2 * D       # per-core qkv proj feats = 384
ROPE_THETA = 500000.0
SCALE = float(D) ** -0.5

_CACHE = {}


def _ensure_trace_hooks():
    """Register the NTFF profiling hook that the stub antenv package lacks."""
    if "antenv.axon_hooks" in sys.modules:
        return
    try:
        import antenv
    except ImportError:
        return
    hooks = types.ModuleType("antenv.axon_hooks")
    holder = [None]
    hooks.set_axon_ntff_profile_hook = lambda h: holder.__setitem__(0, h)
    hooks.get_axon_ntff_profile_hook = lambda: holder[0]
    antenv.axon_hooks = hooks
    sys.modules["antenv.axon_hooks"] = hooks
    try:
        from trn_agent_boot.trn_boot import _ntff_profile_via_ctypes

        hook = _ntff_profile_via_ctypes("/opt/axon/libaxon_pjrt.so")
        if hook is not None:
            hooks.set_axon_ntff_profile_hook(hook)
    except Exception:
        pass


def _build():
    from contextlib import ExitStack

    from concourse import bacc
    import concourse.mybir as mybir
    import concourse.tile as tile
    from concourse.bass import ts
    from concourse.tile import add_dep_helper

    f32 = mybir.dt.float32
    bf16 = mybir.dt.bfloat16
    AF = mybir.ActivationFunctionType
    OP = mybir.AluOpType

    # stream_shuffle permutes partitions WITHIN each 32-partition quadrant
    # (mask[i] = source partition within the quadrant). The rope pairs are
    # interleaved host-side (per-head order [x1a, x2a, x1b, x2b] in 16-blocks)
    # so the rotate-half partner always sits 16 partitions away in the same
    # quadrant: out[p] = in[p ^ 16].
    SHUF_MASK = [i ^ 16 for i in range(32)]
    KO = HID // 128           # 16 contraction chunks
    NQ = T // 512             # 4 seq chunks of 512
    NB = T // 128             # 16 k blocks of 128

    nc = bacc.Bacc("TRN2", target_bir_lowering=False, debug=False, num_devices=NCORES)

    hT = nc.dram_tensor("hT", [T // 512, 128, HID // 128, 512], bf16, kind="ExternalInput")
    # weights pre-tiled host-side to [128 partitions, KO * feat] so a
    # multi-k-chunk DMA reads contiguous multi-KB lines per partition
    wT = nc.dram_tensor("wT", [128, KO * QKV_F], bf16, kind="ExternalInput")
    cosf = nc.dram_tensor("cosf", [128, T], bf16, kind="ExternalInput")
    sinf = nc.dram_tensor("sinf", [128, T], bf16, kind="ExternalInput")
    # perm / ident / tri / ones packed into one tensor = one DMA trigger
    misc = nc.dram_tensor("misc", [128, 4, 128], bf16, kind="ExternalInput")
    woT = nc.dram_tensor("woT", [128, KO * HID], bf16, kind="ExternalInput")
    out = nc.dram_tensor("out", [SPC, HID], f32, kind="ExternalOutput")
    # one AllToAll per head: [dst core, head feats, seq chunk]
    a2a_in = [
        nc.dram_tensor(f"a2a_in{g}", [NCORES, D, SPC], bf16) for g in range(HPC)
    ]
    a2a_out = [
        nc.dram_tensor(f"a2a_out{g}", [NCORES, D, SPC], bf16) for g in range(HPC)
    ]

    with tile.TileContext(nc) as tc, ExitStack() as ctx:
        consts = ctx.enter_context(tc.tile_pool(name="consts", bufs=1))
        persist = ctx.enter_context(tc.tile_pool(name="persist", bufs=1))

        # first QKV matmuls are gated on wt + the first hT chunk; load those
        # first, finest-grained up front so compute starts as early as
        # possible, coarser (fatter-lined, fewer triggers) after
        wt_t = consts.tile([128, KO, QKV_F], bf16, tag="wt")
        ht0_t = consts.tile([128, KO, 512], bf16, tag="ht0")
        nc.sync.dma_start(wt_t[:, 0, :], wT.ap()[:, 0:QKV_F])
        # first hT chunk rides the (otherwise idle) scalar queue so the two
        # gating transfers start in parallel instead of trigger-serialized
        nc.scalar.dma_start(ht0_t[:, 0:1, :], hT.ap()[0, :, 0:1, :])
        nc.sync.dma_start(wt_t[:, 1:4, :], wT.ap()[:, QKV_F:4 * QKV_F])
        nc.sync.dma_start(ht0_t[:, 1:4, :], hT.ap()[0, :, 1:4, :])
        nc.sync.dma_start(wt_t[:, 4:8, :], wT.ap()[:, 4 * QKV_F:8 * QKV_F])
        nc.sync.dma_start(ht0_t[:, 4:8, :], hT.ap()[0, :, 4:8, :])
        nc.sync.dma_start(wt_t[:, 8:16, :], wT.ap()[:, 8 * QKV_F:16 * QKV_F])
        nc.sync.dma_start(ht0_t[:, 8:12, :], hT.ap()[0, :, 8:12, :])
        nc.sync.dma_start(ht0_t[:, 12:16, :], hT.ap()[0, :, 12:16, :])
        cos_t = consts.tile([128, T], bf16, tag="cos")
        nc.sync.dma_start(cos_t, cosf.ap())
        sin_t = consts.tile([128, T], bf16, tag="sin")
        nc.sync.dma_start(sin_t, sinf.ap())
        misc_t = consts.tile([128, 4, 128], bf16, tag="misc")
        nc.sync.dma_start(misc_t, misc.ap())
        # hT chunks n=1..3 trigger up-front (3 pool bufs, so no buffer-free
        # waits) — in the n-loop they would queue behind the rope-gated
        # k2/vlo copies and head-of-line block the input feed
        ht_tiles = [None] * NQ
        perm_t = misc_t[:, 0, :]
        ident_t = misc_t[:, 1, :]
        tri_t = misc_t[:, 2, :]
        ones_t = misc_t[:, 3, :]

        # Persistent activation tiles (live across phases A/B). q_t[m] holds
        # heads 2m (partitions 0:64) and 2m+1 (64:128); scores for the even
        # head contract against k_t (k at 0:64, zeros at 64:128), the odd head
        # against k2_t (zeros at 0:64, k at 64:128) — no per-head q copies.
        q_t = [persist.tile([128, T], bf16, tag=f"q{p}", name=f"q{p}") for p in range(2)]
        k_t = persist.tile([128, T], bf16, tag="kt")
        k2_t = persist.tile([128, T], bf16, tag="k2t")
        vlo_t = persist.tile([64, T], bf16, tag="vlo")
        vaug_t = persist.tile([128, KO, D + 1], bf16, tag="vaug")

        # zero the K-padding rows once (on gpsimd: idle at startup, and the
        # vector engine is on the QKV critical path)
        nc.gpsimd.memset(k_t[64:128, :], 0.0)
        nc.gpsimd.memset(k2_t[0:64, :], 0.0)

        # ---- Phase A: QKV projection + RoPE (outputs transposed [feat, seq]) ----
        with nc.named_scope("qkv"):
            with (
                tc.tile_pool(name="htp", bufs=3) as ht_pool,
                tc.tile_pool(name="atmp", bufs=3) as atmp,
                tc.tile_pool(name="psA", bufs=6, space="PSUM") as psA,
                tc.tile_pool(name="psV", bufs=2, space="PSUM") as psV,
            ):
                nc.gpsimd.memset(vaug_t[:, :, D:D + 1], 1.0)
                ht_tiles[0] = ht0_t
                for n in range(1, NQ):
                    ht_tiles[n] = ht_pool.tile(
                        [128, KO, 512], bf16, tag="ht", name=f"ht{n}"
                    )
                    for k4 in range(4):
                        nc.sync.dma_start(
                            ht_tiles[n][:, ts(k4, 4), :], hT.ap()[n, :, ts(k4, 4), :]
                        )
                for n in range(NQ):
                    ht_t = ht_tiles[n]
                    pending = None
                    for m in range(3):
                        pq = psA.tile([128, 512], f32, tag="pq")
                        for k in range(KO):
                            nc.tensor.matmul(
                                pq,
                                wt_t[:, k, ts(m, 128)],
                                ht_t[:, k, :],
                                start=(k == 0),
                                stop=(k == KO - 1),
                            )
                        # PSUM->SBUF cast on the scalar engine: it is idle all
                        # through QKV, while the vector queue carries the rope
                        # chain — keeps psA recycling off the vector backlog
                        xb = atmp.tile([128, 512], bf16, tag="xb")
                        nc.scalar.copy(xb, pq)
                        if pending is not None:
                            pending()
                            pending = None
                        if m < 2:
                            def rope_q(m=m, n=n, xb=xb):
                                # two q heads: rotate-half via DVE stream_shuffle
                                # (swaps 32-partition blocks within each 64)
                                psh = atmp.tile([128, 512], bf16, tag="psh", name="psh")
                                nc.vector.stream_shuffle(psh, xb, SHUF_MASK)
                                t1 = atmp.tile([128, 512], f32, tag="t1", name="t1")
                                nc.vector.tensor_tensor(t1, xb, cos_t[:, ts(n, 512)], OP.mult)
                                t2 = atmp.tile([128, 512], f32, tag="t2", name="t2")
                                nc.vector.tensor_tensor(t2, psh, sin_t[:, ts(n, 512)], OP.mult)
                                nc.vector.tensor_tensor(q_t[m][:, ts(n, 512)], t1, t2, OP.add)
                            pending = rope_q
                        else:
                            def rope_kv(n=n, xb=xb):
                                # k head on partitions 0:64 (rope), v on 64:128
                                psh = atmp.tile([128, 512], bf16, tag="psh", name="psh")
                                nc.vector.stream_shuffle(psh, xb, SHUF_MASK)
                                t1 = atmp.tile([128, 512], f32, tag="t1", name="t1")
                                nc.vector.tensor_tensor(
                                    t1[0:64, :], xb[0:64, :], cos_t[0:64, ts(n, 512)], OP.mult
                                )
                                t2 = atmp.tile([128, 512], f32, tag="t2", name="t2")
                                nc.vector.tensor_tensor(
                                    t2[0:64, :], psh[0:64, :], sin_t[0:64, ts(n, 512)], OP.mult
                                )
                                nc.vector.tensor_tensor(
                                    k_t[0:64, ts(n, 512)], t1[0:64, :], t2[0:64, :], OP.add
                                )
                                # odd heads' k copy (partition remap via DMA)
                                nc.sync.dma_start(
                                    k2_t[64:128, ts(n, 512)], k_t[0:64, ts(n, 512)]
                                )
                                # v slice to partitions 0:64 via DMA (partition remap)
                                nc.sync.dma_start(vlo_t[:, ts(n, 512)], xb[64:128, :])
                                for j in range(4 * n, 4 * n + 4):
                                    pv = psV.tile([128, D], bf16, tag="pv", name="pv")
                                    nc.tensor.transpose(
                                        pv, vlo_t[:, ts(j, 128)], ident_t[0:64, 0:64]
                                    )
                                    nc.vector.tensor_copy(vaug_t[:, j, 0:D], pv)
                            pending = rope_kv
                    if pending is not None:
                        pending()



        wo_t = consts.tile([128, KO, HID], bf16, tag="wo")
        lo_t = persist.tile([128, KO, SPC], bf16, tag="lo", name="lo")
        # w_o streams in right after the QKV input traffic drains; it must be
        # ahead of the attention staging writes so the collectives' completion
        # counters never queue behind these 8MB of transfers
        for k4 in range(4):
            nc.sync.dma_start(
                wo_t[:, ts(k4, 4), :], woT.ap()[:, 4 * k4 * HID:4 * (k4 + 1) * HID]
            )

        # ---- Phase B: causal attention, 4 heads, scoresT layout ----
        a2a_dmas = []
        ccs = []

        def _emit_cc(g):
            cc = nc.gpsimd.collective_compute(
                "AllToAll",
                OP.bypass,
                replica_groups=[list(range(NCORES))],
                ins=[a2a_in[g].ap()],
                outs=[a2a_out[g].ap()],
            )
            for gg, dd in a2a_dmas:
                if gg == g:
                    add_dep_helper(cc.ins, dd.ins, sync=True, reason="cc waits a2a stage-in")
            return cc

        def _emit_lo(g):
            # o_proj k-chunks stream in as soon as the collective lands. These
            # ride the GPSIMD software-DGE queue: it is already serialized
            # with the collectives, so waiting on cc_g there never blocks the
            # staging/probs pipelines (the sync + vector queues would suffer
            # head-of-line blocking instead). Two batched triggers per head
            # (even/odd src cores) instead of four.
            v = a2a_out[g].ap().rearrange("(j two) p s -> two p j s", two=2)
            for half in range(2):
                dl = nc.gpsimd.dma_start(
                    lo_t[64 * half:64 * half + 64, ts(g, 4), :], v[half]
                )
                add_dep_helper(dl.ins, ccs[g].ins, sync=True, reason="o_proj waits AllToAll")

        with nc.named_scope("attn"):
            with (
                tc.tile_pool(name="probs", bufs=2) as probs_pool,
                tc.tile_pool(name="btmp", bufs=4) as btmp,
                tc.tile_pool(name="psS", bufs=2, space="PSUM") as psS,
                tc.tile_pool(name="psO", bufs=2, space="PSUM") as psO,
                tc.tile_pool(name="psB", bufs=2, space="PSUM") as psB,
            ):
                # the normalize chain (dbc copy -> pb broadcast matmul ->
                # recip -> oth -> staging) of chunk (h, i) is emitted DEFERRED,
                # two score-chunks into the next (h, i): the pb matmul rides
                # the in-order PE queue, so emitted inline it head-of-line
                # blocks the next chunk's scores while the vector engine
                # finishes dbc. Collective/lo emission moves with it.
                pending_norm = [None]
                cc_done = [False] * HPC

                def flush_norm():
                    if pending_norm[0] is not None:
                        pending_norm[0]()
                        pending_norm[0] = None

                def emit_cc_and_lo(g):
                    ccs.append(_emit_cc(g))
                    cc_done[g] = True
                    if g <= 1:
                        _emit_lo(g)
                    elif g == 3:
                        _emit_lo(2)
                        _emit_lo(3)

                for h in range(HPC):
                    for i in range(NQ):
                        nj = 4 * i + 4
                        pr = probs_pool.tile([128, NB, 512], bf16, tag="pr")
                        po = psO.tile([D + 1, 512], f32, tag="po")
                        # every adjacent block pair shares one psum tile + exp
                        # call (the exp over a diagonal pair spans from the
                        # earlier block's causal offset; the later block's
                        # columns below its own offset hold exp(stale psum) —
                        # bounded, finite, and never read by P@V). P@V is
                        # lagged two chunks behind the scores so the PE never
                        # stalls on the exp chain.
                        chunks = [[j, j + 1] for j in range(0, nj, 2)]

                        kh_t = k_t if h % 2 == 0 else k2_t
                        qm_t = q_t[h // 2]

                        def emit_scores(js, i=i, pr=pr, kh_t=kh_t, qm_t=qm_t):
                            offs = [max(0, jj - 4 * i) * 128 for jj in js]
                            pss = psS.tile([128, 2, 512], f32, tag="pss", name="pss")
                            for u in range(2):
                                nc.tensor.matmul(
                                    pss[:, u, offs[u]:512],
                                    kh_t[:, ts(js[u], 128)],
                                    qm_t[:, i * 512 + offs[u]:(i + 1) * 512],
                                    start=True, stop=True,
                                )
                            nc.scalar.activation(
                                pr[:, js[0]:js[0] + 2, offs[0]:512],
                                pss[:, :, offs[0]:512],
                                AF.Exp, scale=SCALE,
                            )
                            for u in range(2):
                                r = js[u] - 4 * i
                                if r >= 0:  # block overlapping the causal diagonal
                                    off = offs[u]
                                    nc.vector.tensor_tensor(
                                        pr[:, js[u], off:off + 128],
                                        pr[:, js[u], off:off + 128],
                                        tri_t, OP.mult,
                                    )

                        def emit_pv(js, i=i, pr=pr, po=po, nj=nj):
                            for jj in js:
                                off = max(0, jj - 4 * i) * 128
                                nc.tensor.matmul(
                                    po[:, off:512], vaug_t[:, jj, :], pr[:, jj, off:512],
                                    start=(jj == 0), stop=(jj == nj - 1),
                                )

                        LAG = 2
                        for ci, ch in enumerate(chunks):
                            emit_scores(ch)
                            if ci == 1:
                                flush_norm()
                                if i == 0 and h > 0 and not cc_done[h - 1]:
                                    emit_cc_and_lo(h - 1)
                            if ci >= LAG:
                                emit_pv(chunks[ci - LAG])
                        for ci in range(max(0, len(chunks) - LAG), len(chunks)):
                            emit_pv(chunks[ci])

                        def norm(h=h, i=i, po=po):
                            # normalize: oT[f, q] = po[f, q] / den[q]; den row
                            # broadcast across partitions via a K=1 ones
                            # matmul, then 1/x on DVE
                            dbc = btmp.tile([D + 1, 512], bf16, tag="dbc")
                            nc.vector.tensor_copy(dbc[D:D + 1, :], po[D:D + 1, :])
                            pb = psB.tile([D, 512], f32, tag="pb")
                            nc.tensor.matmul(
                                pb, ones_t[D:D + 1, 0:D], dbc[D:D + 1, :],
                                start=True, stop=True,
                            )
                            rbs = btmp.tile([D, 512], f32, tag="rbs")
                            nc.vector.reciprocal_approx_fast(out=rbs, in_=pb)
                            oth = btmp.tile([D, 512], bf16, tag="oth")
                            nc.vector.tensor_tensor(oth, po[0:D, :], rbs, OP.mult)
                            # the two staging triggers ride different queues
                            # (sync + gpsimd) so they fire in parallel and the
                            # head's collective trigger sees staging complete
                            # ~0.6us earlier
                            dd = nc.sync.dma_start(
                                a2a_in[h].ap()[2 * i, :, :], oth[:, ts(0, 256)]
                            )
                            a2a_dmas.append((h, dd))
                            dd = nc.gpsimd.dma_start(
                                a2a_in[h].ap()[2 * i + 1, :, :], oth[:, ts(1, 256)]
                            )
                            a2a_dmas.append((h, dd))

                        pending_norm[0] = norm
                flush_norm()
                emit_cc_and_lo(3)

        # ---- Phase D: o_proj for this core's 256 seq rows ----
        # Three k-passes matched to collective arrival: k0..7 (heads 0-1,
        # landed long before attention drains), k8..11 (head 2), k12..15
        # (head 3, the only chunks on the final collective's critical path).
        # Output stores are batched per group pair ([128, 1024] fp32).
        with nc.named_scope("oproj"):
            with (
                tc.tile_pool(name="dtmp", bufs=2) as dtmp,
                tc.tile_pool(name="psD", bufs=1, space="PSUM") as psD,
            ):
                groups = [(m, e4) for m in range(SPC // 128) for e4 in range(HID // 512)]
                psos = [
                    psD.tile([128, 512], f32, tag=f"pso{g}", name=f"pso{g}")
                    for g in range(len(groups))
                ]
                for ka, kb in ((0, 8), (8, 12)):
                    for g, (m, e4) in enumerate(groups):
                        for k in range(ka, kb):
                            nc.tensor.matmul(
                                psos[g],
                                lo_t[:, k, ts(m, 128)],
                                wo_t[:, k, ts(e4, 512)],
                                start=(k == 0),
                                stop=False,
                            )
                ob = None
                for g, (m, e4) in enumerate(groups):
                    for k in range(12, KO):
                        nc.tensor.matmul(
                            psos[g],
                            lo_t[:, k, ts(m, 128)],
                            wo_t[:, k, ts(e4, 512)],
                            start=False,
                            stop=(k == KO - 1),
                        )
                    if g < len(groups) - 2:
                        if g % 2 == 0:
                            ob = dtmp.tile([128, 1024], f32, tag="ob")
                        nc.vector.tensor_copy(ob[:, 512 * (g % 2):512 * (g % 2) + 512], psos[g])
                        if g % 2 == 1:
                            nc.sync.dma_start(
                                out.ap()[ts(m, 128), 1024 * ((e4 - 1) // 2):1024 * ((e4 - 1) // 2) + 1024],
                                ob,
                            )
                    else:
                        # last pair: per-512 stores with the copies split
                        # across scalar+vector so the final transfer starts
                        # one copy earlier and pipelines behind the other
                        obh = dtmp.tile([128, 512], f32, tag="obh")
                        if g % 2 == 0:
                            nc.scalar.copy(obh, psos[g])
                        else:
                            nc.vector.tensor_copy(obh, psos[g])
                        nc.sync.dma_start(
                            out.ap()[ts(m, 128), 512 * e4:512 * e4 + 512], obh
                        )

    nc.compile()
    return nc


def _get_nc():
    if "nc" not in _CACHE:
        _CACHE["nc"] = _build()
    return _CACHE["nc"]


def _host_prep(hidden_states, positions, w_qkv, w_o):
    bf16 = ml_dtypes.bfloat16
    hTb = np.ascontiguousarray(hidden_states.astype(np.float32).T).astype(bf16)
    # pretile to [n, p, ko, s] so each 512-seq chunk is one contiguous DMA
    hTt = np.ascontiguousarray(
        hTb.reshape(HID // 128, 128, T // 512, 512).transpose(2, 1, 0, 3)
    )
    woTb = np.ascontiguousarray(w_o.astype(np.float32).T).astype(bf16)
    # o_proj contraction order matches the per-head AllToAll arrival order:
    # head h of every core, h = 0..3
    rows = np.concatenate(
        [
            (np.arange(NCORES)[:, None] * FPC + h * D + np.arange(D)[None, :]).reshape(-1)
            for h in range(HPC)
        ]
    )
    woTb = woTb[rows]
    # pre-tile to [128, KO * HID]: partition p holds its k-chunks contiguously
    KO = HID // 128
    woTb = np.ascontiguousarray(
        woTb.reshape(KO, 128, HID).transpose(1, 0, 2).reshape(128, KO * HID)
    )

    inv = 1.0 / (ROPE_THETA ** (np.arange(0, D, 2, dtype=np.float32) / D))  # [32]
    ang = positions.astype(np.float32)[:, None] * inv[None, :]              # [T, 32]
    cos = np.cos(ang).T  # [32, T]
    sin = np.sin(ang).T
    # rope-pair interleave: position p holds original feature P64[p % 64]
    P64 = np.r_[0:16, 32:48, 16:32, 48:64]
    p = np.arange(128)
    fr = P64[p % D] % (D // 2)
    sgn = np.where(P64[p % D] < (D // 2), -1.0, 1.0).astype(np.float32)
    cosf = np.ascontiguousarray(cos[fr]).astype(bf16)                 # [128, T]
    sinf = np.ascontiguousarray(sin[fr] * sgn[:, None]).astype(bf16)  # [128, T]

    partner = p ^ 16
    perm = np.zeros((128, 128), dtype=np.float32)
    perm[p, partner] = 1.0
    ident = np.eye(128, dtype=np.float32)
    tri = (np.arange(128)[None, :] >= np.arange(128)[:, None]).astype(np.float32)
    ones_m = np.ones((128, 128), dtype=np.float32)
    misc = np.ascontiguousarray(
        np.stack([perm, ident, tri, ones_m], axis=1)
    ).astype(bf16)  # [128, 4, 128]

    q_size = NH * D
    kv_size = NKV * D
    in_maps = []
    for c in range(NCORES):
        wq = w_qkv[c * FPC:(c + 1) * FPC]
        wq = wq.reshape(HPC, D, HID)[:, P64, :].reshape(FPC, HID)
        wk = w_qkv[q_size + c * D:q_size + (c + 1) * D][P64]
        wv = w_qkv[q_size + kv_size + c * D:q_size + kv_size + (c + 1) * D]
        wTc = np.ascontiguousarray(
            np.concatenate([wq, wk, wv], axis=0).astype(np.float32).T
        ).astype(bf16)  # [HID, QKV_F]
        # pre-tile to [128, KO * QKV_F]
        wTc = np.ascontiguousarray(
            wTc.reshape(KO, 128, QKV_F).transpose(1, 0, 2).reshape(128, KO * QKV_F)
        )
        in_maps.append(
            {
                "hT": hTt,
                "wT": wTc,
                "cosf": cosf,
                "sinf": sinf,
                "misc": misc,
                "woT": woTb,
            }
        )
    return in_maps


def run(inputs, trace=False):
    """Run on 8 NeuronCores; returns (full_output, BassKernelResults)."""
    if trace:
        _ensure_trace_hooks()
    from concourse import bass_utils

    if trace:
        bass_utils.upload_artifacts = lambda tmpdir: tmpdir
    nc = _get_nc()
    in_maps = _host_prep(
        np.asarray(inputs["hidden_states"]),
        np.asarray(inputs["positions"]),
        np.asarray(inputs["w_qkv"]),
        np.asarray(inputs["w_o"]),
    )
    res = bass_utils.run_bass_kernel_spmd(
        nc, in_maps, core_ids=list(range(NCORES)), trace=trace
    )
    full = np.concatenate(
        [res.results[c]["out"] for c in range(NCORES)], axis=0
    ).astype(np.float32)
    return full, res


def kernel(**inputs) -> np.ndarray:
    trace = bool(os.environ.get("KERNEL_TRACE"))
    full, _ = run(inputs, trace=trace)
    return full
